# revision 42
# baseline (speedup 1.0000x reference)
"""Bahdanau (additive) attention kernel for Trainium2, 8 NeuronCores.

Reference computation (per batch b):
    w1q = query @ W1                         # (T, U)
    w2k = value @ W2                         # (S, U)
    scores[t,s] = sum_u scale[u] * tanh(w1q[t,u] + w2k[s,u])
    attn = softmax(scores, axis=-1)          # (T, S)
    context = attn @ value                   # (T, V)

Sharding: batch (B=8) data-parallel, one batch per core. W1/W2/scale replicated.

Score path: instead of materializing the (T,S,U) broadcast sum and running
8.4M tanh evaluations through the ACT engine (~55us roofline), tanh is
expanded in an odd sine series on [-9,9]:

    tanh(z) ~= sum_m b_m sin(2*pi*m*z/L)            (M=11, L=21)

and sin(mw(q+k)) = sin(mwq)cos(mwk) + cos(mwq)sin(mwk) makes the (t,s,u,m)
reduction bilinear: per-harmonic sin/cos matrices of the two projections are
contracted on the tensor engine as 4M fp16 matmuls accumulating into the
(T,S) score PSUM tile.

ACT's Sin spline is only valid on [-pi, pi] and the DVE has no mod/round
ISA, so harmonics are built without any range reduction: ACT evaluates only
the half-angle base (phi = pi*z/L in [-1.4, 1.4]; sin phi directly,
cos phi = sin(pi/2 - phi), both in range) straight from the projection PSUM,
and all higher harmonics come from the Chebyshev three-term ladder

    X_m = D * X_{m-1} - X_{m-2},   D = 2cos(2*pi*z/L) = 4cos^2(phi) - 2

run elementwise in fp16 on the DVE (bf16 noise amplifies through the
ladder past the 2e-2 gate; fp16 lands at ~7e-3 attn relerr). The even
k-side harmonics are not chained at all: they are single-product leaves
on the otherwise-idle GPSIMD/Pool engine (S_2m = 2*S_m*C_m with the 2
folded into the fold coefficients, C_2m = 2*C_m^2 - 1 where the -1 only
shifts each score row by a constant that softmax ignores). b_m*scale[u]
folds into the small q-side operands on the scalar engine (Copy with a
per-partition scale vector), keeping the DVE ladder loop lean.

Softmax has no max-subtraction (|scores| <= sum|b_m|*||scale||_1 ~ 13; exp
stays in fp32 range); the row sum rides the exp's accumulator, the context
matmul uses PE transposes of unnormalized exp in bf16 against the bf16
value tile, and 1/sum folds into the ACT-side evacuating scales.

value^T and query^T are host-prepared layout copies so the projections
start straight off the DMAs; a short tensor-engine warmup chain during the
DMA window avoids the cold p-state penalty on the projection matmuls.

Engine budget per core (cost model): DVE ~26us (ladders, bottleneck),
ACT ~20us (base trig, folds, softmax/ctx scales), PE ~17us (matmuls),
Pool ~16us (leaf products). Measured on HW with the looped differential
harness: ~55us/execution (vs ~92us for the direct tanh-on-ACT design).
"""

import math

import numpy as np

import concourse.tile as tile
from concourse import bacc, mybir
from concourse.bass_utils import run_bass_kernel_spmd
from concourse.masks import make_identity

B, T, S = 8, 64, 512
QU, VU, U = 1024, 512, 256
N_CORES = 8
F32 = mybir.dt.float32
BF16 = mybir.dt.bfloat16
F16 = mybir.dt.float16
AF = mybir.ActivationFunctionType
OP = mybir.AluOpType

M_HARM = 11
L_PER = 21.0
FIT_DOM = 9.0
PI = math.pi
# k-side ladder column split: [0, KS) on DVE (2x fp16), [KS, 2S) on Pool.
KS = 1024


def _fit_sine_coeffs(m_harm=M_HARM, period=L_PER, dom=FIT_DOM):
    """Minimax-ish (Lawson-iterated lstsq) odd sine series for tanh on
    [-dom, dom]; behavior outside the data region is unconstrained."""
    x = np.linspace(0.0, dom, 8001)
    t = np.tanh(x)
    A = np.stack([np.sin(2 * np.pi * m * x / period) for m in range(1, m_harm + 1)], 1)
    w = np.ones_like(x)
    c = None
    for _ in range(60):
        c = np.linalg.lstsq(A * w[:, None], t * w, rcond=None)[0]
        r = np.abs(A @ c - t)
        w *= (1e-12 + r) ** 0.5
        w /= w.max()
    return [float(v) for v in c]


B_COEF = _fit_sine_coeffs()


def build_program(iters=1):
    """Build the kernel program. With iters > 1 the entire body (input DMAs
    through output DMAs) runs inside a hardware loop — used by the timing
    harness to measure per-execution HW time with the dispatch overhead
    amortized over many executions."""
    import contextlib

    nc = bacc.Bacc(
        "TRN2",
        target_bir_lowering=False,
        debug=False,
        enable_asserts=False,
        num_devices=N_CORES,
    )
    qt_d = nc.dram_tensor("qt", (QU, T), BF16, kind="ExternalInput").ap()
    v_d = nc.dram_tensor("value", (S, VU), BF16, kind="ExternalInput").ap()
    vt_d = nc.dram_tensor("vt", (VU, S), BF16, kind="ExternalInput").ap()
    w1_d = nc.dram_tensor("w1", (QU, U), BF16, kind="ExternalInput").ap()
    w2_d = nc.dram_tensor("w2", (VU, U), BF16, kind="ExternalInput").ap()
    sc_d = nc.dram_tensor("scale", (U, 1), F32, kind="ExternalInput").ap()
    ctx_d = nc.dram_tensor("context", (T, VU), F32, kind="ExternalOutput").ap()
    att_d = nc.dram_tensor("attn", (T, S), F32, kind="ExternalOutput").ap()

    with tile.TileContext(nc) as tc:
        with (
            tc.tile_pool(name="const", bufs=1) as cpool,
            tc.tile_pool(name="ktmp", bufs=3) as ktmpp,
            tc.tile_pool(name="qtmp", bufs=3) as qtmpp,
            tc.tile_pool(name="smx", bufs=1) as smxp,
            tc.tile_pool(name="ps_tr", bufs=1, space="PSUM") as ps_tr,
            tc.tile_pool(name="ps_proj", bufs=3, space="PSUM") as ps_proj,
            tc.tile_pool(name="ps_sc", bufs=1, space="PSUM") as ps_sc,
            tc.tile_pool(name="ps_ctx", bufs=1, space="PSUM") as ps_ctx,
            (tc.For_i(0, iters) if iters > 1 else contextlib.nullcontext()),
        ):
            # ---- input DMAs ----
            # Projections gate everything: vT/W2 feed the k-side, qT/W1 the
            # q-side; value (context matmul only) arrives last.
            vt_sb = cpool.tile([128, 4 * S], BF16, tag="vt")  # [128 v, 4d * 512s]
            vt = vt_sb.rearrange("p (d s) -> p d s", d=4)
            nc.sync.dma_start(
                out=vt[:, 0:2],
                in_=vt_d.rearrange("(d p) s -> p d s", p=128)[:, 0:2],
            )
            w2_sb = cpool.tile([128, 4 * U], BF16, tag="w2")
            nc.scalar.dma_start(
                out=w2_sb.rearrange("p (c u) -> p c u", c=4),
                in_=w2_d.rearrange("(c p) u -> p c u", p=128),
            )
            nc.sync.dma_start(
                out=vt[:, 2:4],
                in_=vt_d.rearrange("(d p) s -> p d s", p=128)[:, 2:4],
            )
            qT_sb = cpool.tile([128, 8 * T], BF16, tag="qT")
            nc.scalar.dma_start(
                out=qT_sb.rearrange("p (c t) -> p c t", c=8),
                in_=qt_d.rearrange("(c p) t -> p c t", p=128),
            )
            w1_sb = cpool.tile([128, 8 * U], BF16, tag="w1")
            nc.sync.dma_start(
                out=w1_sb.rearrange("p (c u) -> p c u", c=8),
                in_=w1_d.rearrange("(c p) u -> p c u", p=128),
            )
            sc_sb = cpool.tile([128, 2], F32, tag="sc")
            nc.gpsimd.dma_start(
                out=sc_sb.rearrange("p c -> p c ()"),
                in_=sc_d.rearrange("(c p) x -> p c x", p=128),
            )
            v_sb = cpool.tile([128, 4 * VU], BF16, tag="v")
            nc.gpsimd.dma_start(
                out=v_sb.rearrange("p (c v) -> p c v", c=4),
                in_=v_d.rearrange("(c p) v -> p c v", p=128),
            )

            # ---- constants ----
            ident_f = cpool.tile([T, T], F32, tag="identf")
            make_identity(nc, ident_f)
            zero_b = cpool.tile([128, 1], F32, tag="zerob")
            nc.vector.memset(zero_b, 0.0)
            pihalf = cpool.tile([128, 1], F32, tag="pihalf")
            nc.vector.memset(pihalf, PI / 2)
            # b_m-scaled copies of the scale vector for the q-side folds
            scb = cpool.tile([128, 2 * M_HARM], F32, tag="scb")
            for m in range(1, M_HARM + 1):
                coef = B_COEF[m - 1] * (2.0 if m in (6, 8, 10) else 1.0)
                nc.vector.tensor_scalar_mul(
                    scb[:, 2 * (m - 1) : 2 * m], sc_sb, coef
                )
            # PE p-state warmup during the DMA window
            wu_l = cpool.tile([128, 128], BF16, tag="wul")
            nc.vector.memset(wu_l, 0.0)
            wu_r = cpool.tile([128, 512], BF16, tag="wur")
            nc.vector.memset(wu_r, 0.0)
            wu_ps = ps_tr.tile([128, 512], F32, tag="wups")
            for i in range(4):
                nc.tensor.matmul(wu_ps, lhsT=wu_l, rhs=wu_r, start=True, stop=True)

            # ---- projections (stay in PSUM; ACT base reads them directly) ----
            ps2 = [
                ps_proj.tile([128, S], F32, tag="pj", name=f"psw2k_{uh}")
                for uh in range(2)
            ]
            for uh in range(2):
                for c in range(4):
                    nc.tensor.matmul(
                        ps2[uh],
                        lhsT=w2_sb.rearrange("p (c u) -> p c u", c=4)[
                            :, c, 128 * uh : 128 * (uh + 1)
                        ],
                        rhs=vt[:, c, :],
                        start=(c == 0),
                        stop=(c == 3),
                    )
            ps1 = ps_proj.tile([128, 2 * T], F32, tag="pj", name="psw1q")
            for uh in range(2):
                for c in range(8):
                    nc.tensor.matmul(
                        ps1[:, T * uh : T * (uh + 1)],
                        lhsT=w1_sb.rearrange("p (c u) -> p c u", c=8)[
                            :, c, 128 * uh : 128 * (uh + 1)
                        ],
                        rhs=qT_sb[:, T * c : T * (c + 1)],
                        start=(c == 0),
                        stop=(c == 7),
                    )

            # ---- half-angle trig base (ACT, straight from PSUM) ----
            # phi = pi*z/L in [-1.4, 1.4]: sin(phi) directly, cos(phi) as
            # sin(pi/2 - phi); both arguments stay inside Sin's [-pi, pi].
            # sh = sin(pi*z/L) (half angle, for D = 2 - 4*sh^2) and the
            # first harmonic S1 = sin(2*pi*z/L) directly (|2*pi*z/L| < pi).
            PHS = PI / L_PER
            sh_k = cpool.tile([128, 2 * S], F16, tag="shk")
            s1_k = cpool.tile([128, 2 * S], F16, tag="s1k")
            for uh in range(2):
                sl = slice(S * uh, S * (uh + 1))
                nc.scalar.activation(
                    sh_k[:, sl], ps2[uh], AF.Sin, bias=zero_b[:, 0:1], scale=PHS
                )
                nc.scalar.activation(
                    s1_k[:, sl], ps2[uh], AF.Sin, bias=zero_b[:, 0:1], scale=2 * PHS
                )
            sh_q = cpool.tile([128, 2 * T], F16, tag="shq")
            s1_q = cpool.tile([128, 2 * T], F16, tag="s1q")
            for uh in range(2):
                sl = slice(T * uh, T * (uh + 1))
                nc.scalar.activation(
                    sh_q[:, sl], ps1[:, sl], AF.Sin, bias=zero_b[:, 0:1], scale=PHS
                )
                nc.scalar.activation(
                    s1_q[:, sl], ps1[:, sl], AF.Sin, bias=zero_b[:, 0:1], scale=2 * PHS
                )

            # prefetch the exp table: ACT is done with Sin after the base
            # passes (dep on the last pass's output forces the order), so the
            # only sin->exp table switch happens here, off the critical path.
            dummy_e = cpool.tile([128, 1], F32, tag="dummye")
            nc.scalar.activation(
                dummy_e, s1_q[:, 2 * T - 1 : 2 * T], AF.Exp, bias=zero_b[:, 0:1]
            )

            # ---- ladder bases: D = 2 - 4*sh^2 = 2cos(2w z), X_1, X_2,
            # and the stride-2 multiplier D2 = D^2 - 2 = 2cos(4w z) ----
            def emit_base(sh, S1, w, tmpp):
                D = cpool.tile([128, w], F16, tag=f"D{w}", name=f"D_{w}")
                t4 = tmpp.tile([128, w], F16, tag="tmp", name=f"t4_{w}")
                nc.vector.scalar_tensor_tensor(t4, sh, -4.0, sh, OP.mult, OP.mult)
                nc.vector.tensor_scalar(D, t4, 2.0, None, OP.add)
                C1 = cpool.tile([128, w], F16, tag=f"C1{w}", name=f"C1_{w}")
                nc.vector.tensor_scalar(C1, D, 0.5, None, OP.mult)
                S2 = cpool.tile([128, w], F16, tag=f"S2{w}", name=f"S2_{w}")
                nc.vector.tensor_tensor(out=S2, in0=D, in1=S1, op=OP.mult)
                C2 = cpool.tile([128, w], F16, tag=f"C2{w}", name=f"C2_{w}")
                t5 = tmpp.tile([128, w], F16, tag="tmp", name=f"t5_{w}")
                nc.vector.scalar_tensor_tensor(t5, D, 0.5, D, OP.mult, OP.mult)
                nc.vector.tensor_scalar(C2, t5, 1.0, None, OP.subtract)
                D2 = cpool.tile([128, w], F16, tag=f"DD{w}", name=f"D2_{w}")
                t6 = tmpp.tile([128, w], F16, tag="tmp", name=f"t6_{w}")
                nc.vector.scalar_tensor_tensor(t6, D, 1.0, D, OP.mult, OP.mult)
                nc.vector.tensor_scalar(D2, t6, 2.0, None, OP.subtract)
                return D, D2, [None, S1, S2], [None, C1, C2]

            Dk, D2k, Sk, Ck = emit_base(sh_k, s1_k, 2 * S, ktmpp)
            Dq, D2q, Sq, Cq = emit_base(sh_q, s1_q, 2 * T, qtmpp)
            # prefetch the exp table set: ACT is done with Sin after the base
            # passes, so a throwaway exp here pulls the (only) table switch
            # out of the softmax tail.


            # ---- harmonic loop: ladder + fold + score matmuls ----
            scores_ps = ps_sc.tile([T, S], F32, tag="scores")
            n_mm = 0

            def emit_fold_and_matmuls(m):
                nonlocal n_mm
                sq_w = qtmpp.tile([128, 2 * T], F16, tag="qw", name=f"sqw_{m}")
                cq_w = qtmpp.tile([128, 2 * T], F16, tag="qw", name=f"cqw_{m}")
                for uh in range(2):
                    sl = slice(T * uh, T * (uh + 1))
                    mcol = slice(2 * (m - 1) + uh, 2 * (m - 1) + uh + 1)
                    nc.scalar.mul(sq_w[:, sl], Sq[m][:, sl], scb[:, mcol])
                    nc.scalar.mul(cq_w[:, sl], Cq[m][:, sl], scb[:, mcol])
                for uh in range(2):
                    tsl = slice(T * uh, T * (uh + 1))
                    ssl = slice(S * uh, S * (uh + 1))
                    nc.tensor.matmul(
                        scores_ps,
                        lhsT=sq_w[:, tsl],
                        rhs=Ck[m][:, ssl],
                        start=(n_mm == 0),
                        stop=(n_mm == 4 * M_HARM - 1),
                    )
                    n_mm += 1
                    nc.tensor.matmul(
                        scores_ps,
                        lhsT=cq_w[:, tsl],
                        rhs=Sk[m][:, ssl],
                        start=(n_mm == 0),
                        stop=(n_mm == 4 * M_HARM - 1),
                    )
                    n_mm += 1

            emit_fold_and_matmuls(1)
            emit_fold_and_matmuls(2)

            # Stride-2 seeds (four independent chains per side):
            #   X_3 = (D2 +/- 1) * X_1   (S: +, via S_{-1} = -S_1; C: -)
            #   S_4 = D2 * S_2 (S_0 = 0),  C_4 = D2 * C_2 - 1 (C_0 = 1)
            # Engine assignment (whole tiles; column splits serialize on the
            # tile-granular dependency tracker): the k-side C-even chain
            # (C4 -> C6 -> C8 -> C10) runs on the otherwise-idle Pool engine,
            # everything else on the DVE.
            def emit_seed3(chain, D2, w, sgn_add, nm):
                new = cpool.tile([128, w], F16, tag=nm, name=nm)
                nc.vector.scalar_tensor_tensor(
                    new, D2, 1.0, chain[1],
                    OP.add if sgn_add else OP.subtract, OP.mult,
                )
                chain.append(new)

            emit_seed3(Sk, D2k, 2 * S, True, "kS3")
            emit_seed3(Ck, D2k, 2 * S, False, "kC3")
            emit_seed3(Sq, D2q, 2 * T, True, "qS3")
            emit_seed3(Cq, D2q, 2 * T, False, "qC3")
            emit_fold_and_matmuls(3)

            def emit_seed4(chain, D2, tmpp, w, is_cos, on_pool, nm):
                new = cpool.tile([128, w], F16, tag=nm, name=nm)
                tt = nc.gpsimd.tensor_tensor if on_pool else nc.vector.tensor_tensor
                if not is_cos:
                    tt(out=new, in0=D2, in1=chain[2], op=OP.mult)
                else:
                    tmp = tmpp.tile([128, w], F16, tag="tmp", name=f"t_{nm}")
                    tt(out=tmp, in0=D2, in1=chain[2], op=OP.mult)
                    nc.vector.tensor_scalar(new, tmp, 1.0, None, OP.subtract)
                chain.append(new)

            emit_seed4(Sk, D2k, ktmpp, 2 * S, False, False, "kS4")
            emit_seed4(Ck, D2k, ktmpp, 2 * S, True, False, "kC4")
            emit_seed4(Sq, D2q, qtmpp, 2 * T, False, False, "qS4")
            emit_seed4(Cq, D2q, qtmpp, 2 * T, True, False, "qC4")
            emit_fold_and_matmuls(4)

            def emit_k_chain(m):
                for chain in (Sk, Ck):
                    cn = "S" if chain is Sk else "C"
                    tmp = ktmpp.tile([128, 2 * S], F16, tag="tmp", name=f"kt{cn}_{m}")
                    nc.vector.tensor_tensor(
                        out=tmp, in0=D2k, in1=chain[m - 2], op=OP.mult
                    )
                    new = cpool.tile(
                        [128, 2 * S], F16, tag=f"k{cn}{m}", name=f"k{cn}_{m}"
                    )
                    nc.vector.tensor_tensor(
                        out=new, in0=tmp, in1=chain[m - 4], op=OP.subtract
                    )
                    chain.append(new)

            def emit_k_leaf(m):
                hm = m // 2
                sl = cpool.tile([128, 2 * S], F16, tag=f"kS{m}", name=f"kS_{m}")
                nc.gpsimd.tensor_tensor(out=sl, in0=Sk[hm], in1=Ck[hm], op=OP.mult)
                Sk.append(sl)
                clf = cpool.tile([128, 2 * S], F16, tag=f"kC{m}", name=f"kC_{m}")
                nc.gpsimd.tensor_tensor(out=clf, in0=Ck[hm], in1=Ck[hm], op=OP.mult)
                Ck.append(clf)

            def emit_q_chain(m):
                for chain in (Sq, Cq):
                    cn = "S" if chain is Sq else "C"
                    tmp = qtmpp.tile([128, 2 * T], F16, tag="tmp", name=f"qt{cn}_{m}")
                    nc.vector.tensor_tensor(
                        out=tmp, in0=D2q, in1=chain[m - 2], op=OP.mult
                    )
                    new = cpool.tile(
                        [128, 2 * T], F16, tag=f"q{cn}{m}", name=f"q{cn}_{m}"
                    )
                    nc.vector.tensor_tensor(
                        out=new, in0=tmp, in1=chain[m - 4], op=OP.subtract
                    )
                    chain.append(new)

            emit_k_chain(5)
            emit_q_chain(5)
            emit_fold_and_matmuls(5)
            emit_k_leaf(6)
            emit_q_chain(6)
            emit_fold_and_matmuls(6)
            emit_k_chain(7)
            emit_q_chain(7)
            emit_fold_and_matmuls(7)
            emit_k_leaf(8)
            emit_q_chain(8)
            emit_fold_and_matmuls(8)
            emit_k_chain(9)
            emit_q_chain(9)
            emit_fold_and_matmuls(9)
            emit_k_leaf(10)
            emit_q_chain(10)
            emit_fold_and_matmuls(10)
            emit_k_chain(11)
            emit_q_chain(11)
            emit_fold_and_matmuls(11)

            # ---- softmax over s (free axis), no max-subtraction ----
            e_sb = smxp.tile([T, S], F32, tag="e")
            ssum = smxp.tile([T, 1], F32, tag="ssum")
            nc.scalar.activation(
                e_sb, scores_ps, AF.Exp, bias=zero_b[0:T, 0:1], accum_out=ssum
            )
            rsum = smxp.tile([T, 1], F32, tag="rsum")
            nc.vector.reciprocal(rsum, ssum)
            attn_sb = smxp.tile([T, S], F32, tag="attn")
            nc.scalar.mul(attn_sb, e_sb, rsum[:, 0:1])
            nc.sync.dma_start(out=att_d, in_=attn_sb)

            # ---- context = attn @ value (unnormalized e, scale at the end) ----
            pte = ps_proj.tile([128, 4 * T], F32, tag="pj", name="pte")
            for c in range(4):
                nc.tensor.transpose(
                    pte[:, T * c : T * (c + 1)],
                    e_sb[:, 128 * c : 128 * (c + 1)],
                    ident_f,
                )
            eT_sb = smxp.tile([128, 4 * T], BF16, tag="eT")
            nc.vector.tensor_copy(eT_sb, pte)
            ctx_ps = ps_ctx.tile([T, VU], F32, tag="ctx")
            ctx_sb = smxp.tile([T, VU], F32, tag="ctxsb")
            for vh in range(2):
                vsl = slice(VU * vh // 2, VU * (vh + 1) // 2)
                for c in range(4):
                    nc.tensor.matmul(
                        ctx_ps[:, vsl],
                        lhsT=eT_sb[:, T * c : T * (c + 1)],
                        rhs=v_sb.rearrange("p (c v) -> p c v", c=4)[:, c, vsl],
                        start=(c == 0),
                        stop=(c == 3),
                    )
                nc.scalar.mul(ctx_sb[:, vsl], ctx_ps[:, vsl], rsum[:, 0:1])
                nc.sync.dma_start(out=ctx_d[:, vsl], in_=ctx_sb[:, vsl])

    nc.compile()
    return nc


_NC_CACHE = None


def _get_program():
    global _NC_CACHE
    if _NC_CACHE is None:
        _NC_CACHE = build_program()
    return _NC_CACHE


LAST_RESULTS = None


def make_in_maps(query, value, W1, W2, scale):
    import ml_dtypes

    bf = ml_dtypes.bfloat16
    w1 = np.ascontiguousarray(W1).astype(bf)
    w2 = np.ascontiguousarray(W2).astype(bf)
    sc = np.ascontiguousarray(scale, dtype=np.float32).reshape(U, 1)
    qb = np.asarray(query).astype(bf)
    vb = np.asarray(value).astype(bf)
    return [
        {
            "qt": np.ascontiguousarray(qb[b].T),
            "value": np.ascontiguousarray(vb[b]),
            "vt": np.ascontiguousarray(vb[b].T),
            "w1": w1,
            "w2": w2,
            "scale": sc,
        }
        for b in range(B)
    ]


def kernel(query, value, W1, W2, scale):
    global LAST_RESULTS
    nc = _get_program()
    in_maps = make_in_maps(query, value, W1, W2, scale)
    res = run_bass_kernel_spmd(nc, in_maps, core_ids=list(range(N_CORES)))
    LAST_RESULTS = res
    context = np.stack([res.results[b]["context"] for b in range(B)], axis=0)
    attn = np.stack([res.results[b]["attn"] for b in range(B)], axis=0)
    return context.astype(np.float32), attn.astype(np.float32)


def _make_runner(nc, in_maps):
    """jit/shard_map runner for an arbitrary program built by build_program.

    Returns run(), which executes one dispatch across the 8 cores (inputs
    pre-sharded on device, outputs donated) and blocks until complete.
    """
    import jax
    from jax.sharding import Mesh, NamedSharding, PartitionSpec
    from jax.experimental.shard_map import shard_map

    from concourse import bass2jax, mybir as mb

    bass2jax.install_neuronx_cc_hook()

    partition_name = nc.partition_id_tensor.name if nc.partition_id_tensor else None
    in_names, out_names, out_avals, zero_outs = [], [], [], []
    for alloc in nc.m.functions[0].allocations:
        if not isinstance(alloc, mb.MemoryLocationSet):
            continue
        name = alloc.memorylocations[0].name
        if alloc.kind == "ExternalInput":
            if name != partition_name:
                in_names.append(name)
        elif alloc.kind == "ExternalOutput":
            shape = tuple(alloc.tensor_shape)
            dtype = mb.dt.np(alloc.dtype)
            out_avals.append(jax.core.ShapedArray(shape, dtype))
            out_names.append(name)
            zero_outs.append(np.zeros(shape, dtype))
    n_params = len(in_names)
    n_outs = len(out_avals)
    all_in_names = list(in_names) + list(out_names)
    if partition_name is not None:
        all_in_names.append(partition_name)

    def _body(*args):
        operands = list(args)
        if partition_name is not None:
            operands.append(bass2jax.partition_id_tensor())
        return tuple(
            bass2jax._bass_exec_p.bind(
                *operands,
                out_avals=tuple(out_avals),
                in_names=tuple(all_in_names),
                out_names=tuple(out_names),
                lowering_input_output_aliases=(),
                sim_require_finite=True,
                sim_require_nnan=True,
                nc=nc,
            )
        )

    devices = jax.devices()[:N_CORES]
    mesh = Mesh(np.asarray(devices), ("core",))
    donate = tuple(range(n_params, n_params + n_outs))
    sharded = jax.jit(
        shard_map(
            _body,
            mesh=mesh,
            in_specs=(PartitionSpec("core"),) * (n_params + n_outs),
            out_specs=(PartitionSpec("core"),) * n_outs,
            check_rep=False,
        ),
        donate_argnums=donate,
        keep_unused=True,
    )
    spec = NamedSharding(mesh, PartitionSpec("core"))
    concat_in = [
        jax.device_put(
            np.concatenate([np.asarray(in_maps[c][nm]) for c in range(N_CORES)], 0),
            spec,
        )
        for nm in in_names
    ]
    jax.block_until_ready(concat_in)

    def fresh_zeros():
        zs = [
            jax.device_put(np.zeros((N_CORES * z.shape[0], *z.shape[1:]), z.dtype), spec)
            for z in zero_outs
        ]
        jax.block_until_ready(zs)
        return zs

    out = sharded(*concat_in, *fresh_zeros())  # warm-up / compile
    jax.block_until_ready(out)

    def run():
        import time

        zs = fresh_zeros()
        t0 = time.perf_counter()
        o = sharded(*concat_in, *zs)
        jax.block_until_ready(o)
        return (time.perf_counter() - t0) * 1e9

    return run


def bench_ns(query, value, W1, W2, scale, reps=30):
    """Wall-clock the SPMD executable (jitted once, inputs pre-sharded).

    Returns (min_ns, median_ns) per call: dispatch + 8-core execution,
    excluding H2D of inputs and D2H of outputs. Dominated by the axon
    tunnel round-trip (~60-120 ms), not device time.
    """
    nc = _get_program()
    in_maps = make_in_maps(query, value, W1, W2, scale)
    run = _make_runner(nc, in_maps)
    times = sorted(run() for _ in range(reps))
    return times[0], times[len(times) // 2]


def bench_hw_exec_ns(query, value, W1, W2, scale, k1=64, k2=2048, reps=12):
    """Per-execution hardware time via hardware-looped differential timing.

    Builds the same kernel wrapped in an on-device For loop of k1 and k2
    iterations, wall-clocks both dispatches, and reports
    (minT(k2) - minT(k1)) / (k2 - k1): the tunnel/dispatch overhead
    (~60-120 ms, independent of loop count) cancels, leaving the marginal
    per-execution device time including input DMAs, compute, output DMAs
    and the loop's all-engine barrier.
    """
    in_maps = make_in_maps(query, value, W1, W2, scale)
    run1 = _make_runner(build_program(iters=k1), in_maps)
    run2 = _make_runner(build_program(iters=k2), in_maps)
    t1s, t2s = [], []
    for _ in range(reps):
        t1s.append(run1())
        t2s.append(run2())
    t1, t2 = min(t1s), min(t2s)
    return (t2 - t1) / (k2 - k1), t1, t2
